# revision 37
# baseline (speedup 1.0000x reference)
"""Causal self-attention (GQA, QK-RMSNorm, partial RoPE, per-head gain) on 8 TRN2 cores.

Problem: B=4, T=2048, D=512; 8 q heads / 4 kv heads, head_dim 64, rope dims 16.

Sharding: core c handles batch b=c//2 and head-group g=c%2 (4 q heads + 2 kv
heads). Each core computes a partial projection y_part = ctx_g @ w_proj[:, g].T
over its 256 head dims; the host sums the two partials per batch.

On-device layout is fully "transposed" (feature-major) to keep every matmul
full-speed and avoid on-chip transposition of the attention probabilities:
  scores^T[kt, q] = k^T q   (kt on partitions, q on free axis)
  softmax without max-subtraction (|s| <= 40 << 88, exp cannot overflow fp32)
  ctx^T[d, q] accumulated as (v | ones)^T @ P  -- the 64 ones-columns make the
  matmul emit the softmax denominator replicated across 64 partitions for free.
All matmuls run in float32r (~1.5e-4 rel err, 4x faster than fp32 on TRN2).
"""

import numpy as np

import concourse.bass as bass
import concourse.mybir as mybir
import concourse.tile as tile
from concourse import bacc, bass_utils
from concourse.bass import ts
from concourse.masks import make_identity

P = 128
T = 2048
D = 512
NT = T // P          # 16 t-tiles
NQC = T // 512       # 4 query chunks of 512
HD = 64
ROPE_HALF = 8
EPS = float(np.finfo(np.float32).eps)

F32 = mybir.dt.float32
F32R = mybir.dt.float32r
BF16 = mybir.dt.bfloat16
I32 = mybir.dt.int32
AX = mybir.AxisListType
AF = mybir.ActivationFunctionType
ALU = mybir.AluOpType

_CACHE = {}


def _build(phases=(1, 2, 3)):
    nc = bacc.Bacc("TRN2", target_bir_lowering=False, debug=False)

    xT = nc.dram_tensor("xT", [D, T], F32, kind="ExternalInput").ap()
    wqkvT = nc.dram_tensor("wqkvT", [D, 512], F32, kind="ExternalInput").ap()
    wpT = nc.dram_tensor("wpT", [256, 512], F32, kind="ExternalInput").ap()
    cs = nc.dram_tensor("cs", [T, ROPE_HALF], F32, kind="ExternalInput").ap()
    sn = nc.dram_tensor("sn", [T, ROPE_HALF], F32, kind="ExternalInput").ap()
    gsc = nc.dram_tensor("gsc", [6], F32, kind="ExternalInput").ap()
    yT = nc.dram_tensor("yT", [D, T], F32, kind="ExternalOutput").ap()

    with tile.TileContext(nc) as tc:
        with tc.tile_pool(name="persist", bufs=1) as persist:
            # weights/tables first: every front matmul depends on wqkv
            wqkv_sb = persist.tile([P, 4, 512], F32R)
            nc.sync.dma_start(
                wqkv_sb[:], wqkvT.bitcast(F32R).rearrange("(o p) m -> p o m", p=P)
            )
            xt_sb = persist.tile([P, 4, T], F32R)
            xTv = xT.bitcast(F32R).rearrange("(o p) t -> p o t", p=P)
            for it in range(4):
                nc.sync.dma_start(
                    xt_sb[:, :, ts(it, P)], xTv[:, :, ts(it, P)]
                )
            cs_sb = persist.tile([P, NT, ROPE_HALF], F32)
            nc.sync.dma_start(cs_sb[:], cs.rearrange("(n p) f -> p n f", p=P))
            sn_sb = persist.tile([P, NT, ROPE_HALF], F32)
            nc.sync.dma_start(sn_sb[:], sn.rearrange("(n p) f -> p n f", p=P))
            gsc_sb = persist.tile([P, 6], F32)
            nc.sync.dma_start(gsc_sb[:], gsc[None, :].to_broadcast((P, 6)))
            for it in range(4, NT):
                nc.sync.dma_start(
                    xt_sb[:, :, ts(it, P)], xTv[:, :, ts(it, P)]
                )
            wp_sb = persist.tile([P, 2, 512], F32R)
            nc.sync.dma_start(
                wp_sb[:], wpT.bitcast(F32R).rearrange("(o p) m -> p o m", p=P)
            )
            ident = persist.tile([P, P], F32)
            make_identity(nc, ident[:])
            magic_sb = persist.tile([P, 6], I32)
            nc.vector.memset(magic_sb[:], 0x5F3759DF)

            # qTa: head0 (kv0) rows 0:64, head2 (kv1) rows 64:128
            # qTb: head1 (kv0) rows 0:64, head3 (kv1) rows 64:128
            qTa = persist.tile([P, T], F32R)
            qTb = persist.tile([P, T], F32R)
            kT = persist.tile([P, T], F32R)  # kv0 rows 0:64, kv1 rows 64:128
            # v_sb per t-tile columns: [v_kv0(64) | ones(64) | v_kv1(64) | ones(64)]
            # bf16: the AV matmul weight-loads v every block; bf16 enables the
            # PE fast-weight-load path (4x faster LDWEIGHTS on hardware)
            v_sb = persist.tile([P, NT, 256], BF16)
            v4 = v_sb.rearrange("p n (kv c) -> p n kv c", kv=2)
            nc.gpsimd.memset(v4[:, :, :, HD:128], 1.0)
            # ctxA: heads (4g+0, 4g+1); ctxB: heads (4g+2, 4g+3) -- matches wpT rows
            ctxA = persist.tile([P, T], F32R)
            ctxB = persist.tile([P, T], F32R)

            # ---- Software-pipelined schedule, per 512-token chunk qc:
            #   front(qc): QKV matmuls + RMS stats + rsqrt + mulback + RoPE
            #   attention(qc-1) first half   (fills PE/ACT while rope settles)
            #   transposes(qc)
            #   attention(qc-1) second half + projection(qc-1)
            slots = ((0, 0, qTa), (0, 1, qTa), (1, 0, qTb), (1, 1, qTb))
            with (
                tc.tile_pool(name="p1ps", bufs=2, space="PSUM") as p1ps,
                tc.tile_pool(name="p1tr", bufs=1, space="PSUM") as p1tr,
                tc.tile_pool(name="sps", bufs=2, space="PSUM") as sps,
                tc.tile_pool(name="cps", bufs=2, space="PSUM") as cps,
                tc.tile_pool(name="p1sb", bufs=6) as p1sb,
                tc.tile_pool(name="p1sm", bufs=8) as p1sm,
                tc.tile_pool(name="pp", bufs=8) as pp,
                tc.tile_pool(name="lp", bufs=4) as lp,
                tc.tile_pool(name="ysb", bufs=4) as ysb,
            ):

                def emit_front_tile(it):
                    """QKV matmuls, RMS-norm (rsqrt on DVE), RoPE for t-tile."""
                    qkv_ps = p1ps.tile([P, 512], F32, tag="qkv", name="qkv_ps")
                    for dk in range(4):
                        nc.tensor.matmul(
                            qkv_ps[:],
                            xt_sb[:, dk, ts(it, P)],
                            wqkv_sb[:, dk, :],
                            start=(dk == 0),
                            stop=(dk == 3),
                        )
                    # v (no norm): straight into v_sb with the ones-gap layout
                    nc.vector.tensor_copy(
                        v4[:, it, :, 0:HD],
                        qkv_ps[:, 384:512].rearrange("p (kv c) -> p kv c", c=HD),
                    )
                    # RMS stats over each 64-wide head strip (4 q + 2 k)
                    sq = p1sm.tile([P, 384], F32, tag="sq", name="sq")
                    nc.scalar.square(sq[:], qkv_ps[:, 0:384])
                    ms = p1sm.tile([P, 6], F32, tag="ms", name="ms")
                    nc.vector.reduce_sum(
                        ms[:], sq.rearrange("p (s c) -> p s c", c=HD), axis=AX.X
                    )
                    # rstd = rsqrt(ms/64 + eps): int bit-trick + 2 Newton steps,
                    # all on DVE. Keeping Sqrt/Ln off ACT means every ACT
                    # instruction uses the Exp table set (no table reloads).
                    nc.vector.tensor_scalar(
                        ms[:], ms[:], 1.0 / HD, EPS, op0=ALU.mult, op1=ALU.add
                    )
                    y = p1sm.tile([P, 6], F32, tag="y", name="y")
                    nc.vector.tensor_scalar(
                        y.bitcast(I32)[:], ms.bitcast(I32)[:], 1, None,
                        op0=ALU.arith_shift_right,
                    )
                    nc.vector.tensor_tensor(
                        y.bitcast(I32)[:], magic_sb[:], y.bitcast(I32)[:],
                        ALU.subtract,
                    )
                    nt_ = p1sm.tile([P, 6], F32, tag="nt", name="nt_")
                    for _ in range(2):
                        nc.vector.tensor_mul(nt_[:], y[:], y[:])
                        nc.vector.tensor_mul(nt_[:], nt_[:], ms[:])
                        nc.vector.tensor_scalar(
                            nt_[:], nt_[:], -0.5, 1.5, op0=ALU.mult, op1=ALU.add
                        )
                        nc.vector.tensor_mul(y[:], y[:], nt_[:])
                    # fold per-head gain * 1/sqrt(head_dim) into q strips
                    nc.vector.tensor_mul(y[:], y[:], gsc_sb[:])
                    qkn = p1sb.tile([P, 384], F32, tag="qkn", name="qkn")
                    qk3 = qkn.rearrange("p (s c) -> p s c", c=HD)
                    nc.vector.tensor_tensor(
                        qk3[:],
                        qkv_ps[:, 0:384].rearrange("p (s c) -> p s c", c=HD),
                        y[:, :, None].to_broadcast((P, 6, HD)),
                        ALU.mult,
                    )
                    # RoPE on first 16 dims of each strip (gpsimd)
                    x1 = qk3[:, :, 0:ROPE_HALF]
                    x2 = qk3[:, :, ROPE_HALF : 2 * ROPE_HALF]
                    cb = cs_sb[:, it, None, :].to_broadcast((P, 6, ROPE_HALF))
                    snb = sn_sb[:, it, None, :].to_broadcast((P, 6, ROPE_HALF))
                    t1 = p1sm.tile([P, 6, ROPE_HALF], F32, tag="t1", name="t1")
                    t2 = p1sm.tile([P, 6, ROPE_HALF], F32, tag="t2", name="t2")
                    nc.gpsimd.tensor_mul(t1[:], x1, snb)
                    nc.gpsimd.tensor_mul(x1, x1, cb)
                    nc.gpsimd.tensor_mul(t2[:], x2, snb)
                    nc.gpsimd.tensor_add(x1, x1, t2[:])
                    nc.gpsimd.tensor_mul(x2, x2, cb)
                    nc.gpsimd.tensor_sub(x2, x2, t1[:])
                    return qkn

                def emit_transposes(it, qkn):
                    for lo, dst in ((0, qTa), (128, qTb), (256, kT)):
                        tr = p1tr.tile([P, P], F32, tag="tr", name="tr")
                        nc.tensor.transpose(tr[:], qkn[:, lo : lo + P], ident[:])
                        nc.scalar.copy(dst[:, ts(it, P)], tr[:])

                def emit_attention_pair(qc, s_t):
                    """Both kv-halves (s_u=0 base 0, s_u=1 base 64) of one qT
                    tile, interleaved per kt block so the two score matmuls sit
                    adjacent in the PE stream (distinct row-groups -> they run
                    concurrently in the 128x128 array on hardware)."""
                    qT_tile = qTa if s_t == 0 else qTb
                    nkt = 4 * (qc + 1)
                    ctxs = []
                    for s_u in (0, 1):
                        ctxs.append(
                            cps.tile([P, 512], F32, tag=f"ctx{s_u}",
                                     name=f"ctx{s_u}", bufs=1)
                        )
                    for ktb in range(nkt):
                        d = ktb - 4 * qc
                        qlo = max(0, d) * P
                        pbs = []
                        for s_u in (0, 1):
                            kb = HD * s_u
                            q512 = qT_tile[kb : kb + HD, ts(qc, 512)]
                            s_ps = sps.tile([P, 512], F32, tag="s_ps", name="s_ps")
                            nc.tensor.matmul(
                                s_ps[:, qlo:512],
                                kT[kb : kb + HD, ts(ktb, P)],
                                q512[:, qlo:512],
                                start=True,
                                stop=True,
                            )
                            pb = pp.tile([P, 512], BF16, tag="pb", name="pb")
                            nc.scalar.activation(
                                pb[:, qlo:512], s_ps[:, qlo:512], AF.Exp
                            )
                            if d >= 0:
                                # zero the strictly-lower triangle (kt > q)
                                nc.gpsimd.affine_select(
                                    out=pb[:, qlo : qlo + P],
                                    in_=pb[:, qlo : qlo + P],
                                    compare_op=ALU.is_ge,
                                    fill=0.0,
                                    base=0,
                                    pattern=[[1, P]],
                                    channel_multiplier=-1,
                                )
                                if qlo > 0:
                                    nc.gpsimd.memset(pb[:, 0:qlo], 0.0)
                            pbs.append(pb)
                        for s_u in (0, 1):
                            nc.tensor.matmul(
                                ctxs[s_u][:],
                                v_sb[:, ktb, 128 * s_u : 128 * s_u + 128],
                                pbs[s_u][:],
                                start=(ktb == 0),
                                stop=(ktb == nkt - 1),
                            )
                    for s_u in (0, 1):
                        linv = lp.tile([HD, 512], F32, tag="linv", name="linv")
                        nc.vector.reciprocal(linv[:], ctxs[s_u][HD:128, :])
                        dst = ctxA if s_u == 0 else ctxB
                        rb = HD * s_t
                        nc.vector.tensor_tensor(
                            dst[rb : rb + HD, ts(qc, 512)],
                            ctxs[s_u][0:HD, :],
                            linv[:],
                            ALU.mult,
                        )

                def emit_proj(qc):
                    for ot in range(4):
                        y_ps = cps.tile(
                            [P, 512], F32, tag="y_ps", name="y_ps", bufs=1
                        )
                        for mt, src in ((0, ctxA), (1, ctxB)):
                            nc.tensor.matmul(
                                y_ps[:],
                                wp_sb[:, mt, ts(ot, P)],
                                src[:, ts(qc, 512)],
                                start=(mt == 0),
                                stop=(mt == 1),
                            )
                        y_sb = ysb.tile([P, 512], F32, tag="y_sb", name="y_sb")
                        nc.vector.tensor_copy(y_sb[:], y_ps[:])
                        nc.sync.dma_start(
                            yT[ot * P : (ot + 1) * P, qc * 512 : (qc + 1) * 512],
                            y_sb[:],
                        )

                do1 = 1 in phases
                do2 = 2 in phases
                do3 = 3 in phases
                for qc in range(NQC):
                    if do1:
                        qkns = []
                        for it in range(4 * qc, 4 * qc + 4):
                            qkns.append((it, emit_front_tile(it)))
                        for it, qkn in qkns:
                            emit_transposes(it, qkn)
                    if do2:
                        for s_t in (0, 1):
                            emit_attention_pair(qc, s_t)
                        if do3:
                            emit_proj(qc)

    nc.compile()
    return nc


def _host_inputs(x, w_q, w_k, w_v, w_proj, q_gain):
    """Build the 8 per-core input maps."""
    B = x.shape[0]
    inv_freq = 1.0 / (
        10000.0 ** (np.arange(0, 16, 2, dtype=np.float32) / np.float32(16.0))
    )
    freqs = np.outer(np.arange(T, dtype=np.float32), inv_freq)
    cs = np.cos(freqs).astype(np.float32)
    sn = np.sin(freqs).astype(np.float32)

    per_group = []
    for g in range(2):
        heads = [4 * g + 0, 4 * g + 2, 4 * g + 1, 4 * g + 3]
        qrows = np.concatenate([w_q[64 * h : 64 * h + 64] for h in heads], axis=0)
        krows = w_k[128 * g : 128 * g + 128]
        vrows = w_v[128 * g : 128 * g + 128]
        wqkvT = np.ascontiguousarray(
            np.concatenate([qrows, krows, vrows], axis=0).T.astype(np.float32)
        )
        wpT = np.ascontiguousarray(
            w_proj[:, 256 * g : 256 * g + 256].T.astype(np.float32)
        )
        gains = np.array(
            [q_gain[h] * 0.125 for h in heads] + [1.0, 1.0], dtype=np.float32
        )
        per_group.append((wqkvT, wpT, gains))

    in_maps = []
    for c in range(8):
        b, g = c // 2, c % 2
        wqkvT, wpT, gains = per_group[g]
        in_maps.append(
            {
                "xT": np.ascontiguousarray(x[b].T.astype(np.float32)),
                "wqkvT": wqkvT,
                "wpT": wpT,
                "cs": cs,
                "sn": sn,
                "gsc": gains,
            }
        )
    return in_maps


def kernel(x, w_q, w_k, w_v, w_proj, q_gain, _trace=False):
    x = np.asarray(x, dtype=np.float32)
    w_q = np.asarray(w_q, dtype=np.float32)
    w_k = np.asarray(w_k, dtype=np.float32)
    w_v = np.asarray(w_v, dtype=np.float32)
    w_proj = np.asarray(w_proj, dtype=np.float32)
    q_gain = np.asarray(q_gain, dtype=np.float32)

    if "nc" not in _CACHE:
        _CACHE["nc"] = _build()
    nc = _CACHE["nc"]

    in_maps = _host_inputs(x, w_q, w_k, w_v, w_proj, q_gain)
    res = bass_utils.run_bass_kernel_spmd(
        nc, in_maps, core_ids=list(range(8)), trace=_trace
    )
    _CACHE["last_result"] = res

    B = x.shape[0]
    y = np.empty((B, T, D), dtype=np.float32)
    for b in range(B):
        yT = res.results[2 * b]["yT"] + res.results[2 * b + 1]["yT"]
        y[b] = yT.T
    return y


# revision 40
# speedup vs baseline: 15836.5071x; 15836.5071x over previous
"""Causal self-attention (GQA, QK-RMSNorm, partial RoPE, per-head gain) on 8 TRN2 cores.

Problem: B=4, T=2048, D=512; 8 q heads / 4 kv heads, head_dim 64, rope dims 16.

Sharding: core c handles batch b=c//2 and head-group g=c%2 (4 q heads + 2 kv
heads). Each core computes a partial projection y_part = ctx_g @ w_proj[:, g].T
over its 256 head dims; the host sums the two partials per batch.

On-device layout is fully "transposed" (feature-major) to keep every matmul
full-speed and avoid on-chip transposition of the attention probabilities:
  scores^T[kt, q] = k^T q   (kt on partitions, q on free axis)
  softmax without max-subtraction (|s| <= 40 << 88, exp cannot overflow fp32)
  ctx^T[d, q] accumulated as (v | ones)^T @ P  -- the 64 ones-columns make the
  matmul emit the softmax denominator replicated across 64 partitions for free.
Matmuls run in float32r (~1.5e-4 rel err, 4x faster than fp32 on TRN2);
the probabilities/V pair uses bf16, which enables the PE fast-weight-load
path for the per-block V weight loads. rsqrt for the RMS norm runs on the
vector engine (bit-trick + 2 Newton steps) so the scalar engine only ever
uses the Exp activation table (no table-reload stalls).
"""

import numpy as np

import concourse.bass as bass
import concourse.mybir as mybir
import concourse.tile as tile
from concourse import bacc, bass_utils
from concourse.bass import ts
from concourse.masks import make_identity

P = 128
T = 2048
D = 512
NT = T // P          # 16 t-tiles
NQC = T // 512       # 4 query chunks of 512
HD = 64
ROPE_HALF = 8
EPS = float(np.finfo(np.float32).eps)

F32 = mybir.dt.float32
F32R = mybir.dt.float32r
BF16 = mybir.dt.bfloat16
I32 = mybir.dt.int32
AX = mybir.AxisListType
AF = mybir.ActivationFunctionType
ALU = mybir.AluOpType

_CACHE = {}


def _build(phases=(1, 2, 3)):
    nc = bacc.Bacc("TRN2", target_bir_lowering=False, debug=False)

    xT = nc.dram_tensor("xT", [D, T], F32, kind="ExternalInput").ap()
    wqkvT = nc.dram_tensor("wqkvT", [D, 512], F32, kind="ExternalInput").ap()
    wpT = nc.dram_tensor("wpT", [256, 512], F32, kind="ExternalInput").ap()
    cs = nc.dram_tensor("cs", [T, ROPE_HALF], F32, kind="ExternalInput").ap()
    sn = nc.dram_tensor("sn", [T, ROPE_HALF], F32, kind="ExternalInput").ap()
    gsc = nc.dram_tensor("gsc", [6], F32, kind="ExternalInput").ap()
    yT = nc.dram_tensor("yT", [D, T], F32, kind="ExternalOutput").ap()

    with tile.TileContext(nc) as tc:
        with tc.tile_pool(name="persist", bufs=1) as persist:
            # weights/tables first: every front matmul depends on wqkv
            wqkv_sb = persist.tile([P, 4, 512], F32R)
            nc.sync.dma_start(
                wqkv_sb[:], wqkvT.bitcast(F32R).rearrange("(o p) m -> p o m", p=P)
            )
            xt_sb = persist.tile([P, 4, T], F32R)
            xTv = xT.bitcast(F32R).rearrange("(o p) t -> p o t", p=P)
            for it in range(4):
                nc.sync.dma_start(
                    xt_sb[:, :, ts(it, P)], xTv[:, :, ts(it, P)]
                )
            cs_sb = persist.tile([P, NT, ROPE_HALF], F32)
            nc.sync.dma_start(cs_sb[:], cs.rearrange("(n p) f -> p n f", p=P))
            sn_sb = persist.tile([P, NT, ROPE_HALF], F32)
            nc.sync.dma_start(sn_sb[:], sn.rearrange("(n p) f -> p n f", p=P))
            gsc_sb = persist.tile([P, 6], F32)
            nc.sync.dma_start(gsc_sb[:], gsc[None, :].to_broadcast((P, 6)))
            for it in range(4, NT):
                nc.sync.dma_start(
                    xt_sb[:, :, ts(it, P)], xTv[:, :, ts(it, P)]
                )
            wp_sb = persist.tile([P, 2, 512], F32R)
            nc.sync.dma_start(
                wp_sb[:], wpT.bitcast(F32R).rearrange("(o p) m -> p o m", p=P)
            )
            ident = persist.tile([P, P], F32)
            make_identity(nc, ident[:])
            magic_sb = persist.tile([P, 6], I32)
            nc.vector.memset(magic_sb[:], 0x5F3759DF)

            # qTa: head0 (kv0) rows 0:64, head2 (kv1) rows 64:128
            # qTb: head1 (kv0) rows 0:64, head3 (kv1) rows 64:128
            qTa = persist.tile([P, T], F32R)
            qTb = persist.tile([P, T], F32R)
            kT = persist.tile([P, T], F32R)  # kv0 rows 0:64, kv1 rows 64:128
            # v_sb per t-tile columns: [v_kv0(64) | ones(64) | v_kv1(64) | ones(64)]
            # bf16: the AV matmul weight-loads v every block; bf16 enables the
            # PE fast-weight-load path (4x faster LDWEIGHTS on hardware)
            v_sb = persist.tile([P, NT, 256], BF16)
            v4 = v_sb.rearrange("p n (kv c) -> p n kv c", kv=2)
            nc.gpsimd.memset(v4[:, :, :, HD:128], 1.0)
            # ctxA: heads (4g+0, 4g+1); ctxB: heads (4g+2, 4g+3) -- matches wpT rows
            ctxA = persist.tile([P, T], F32R)
            ctxB = persist.tile([P, T], F32R)

            # ---- Software-pipelined schedule, per 512-token chunk qc:
            #   front(qc): QKV matmuls + RMS stats + rsqrt + mulback + RoPE
            #   attention(qc-1) first half   (fills PE/ACT while rope settles)
            #   transposes(qc)
            #   attention(qc-1) second half + projection(qc-1)
            with (
                tc.tile_pool(name="p1ps", bufs=2, space="PSUM") as p1ps,
                tc.tile_pool(name="p1tr", bufs=1, space="PSUM") as p1tr,
                tc.tile_pool(name="sps", bufs=2, space="PSUM") as sps,
                tc.tile_pool(name="cps", bufs=2, space="PSUM") as cps,
                tc.tile_pool(name="p1sb", bufs=6) as p1sb,
                tc.tile_pool(name="p1sm", bufs=8) as p1sm,
                tc.tile_pool(name="pp", bufs=8) as pp,
                tc.tile_pool(name="lp", bufs=4) as lp,
                tc.tile_pool(name="ysb", bufs=4) as ysb,
            ):

                def emit_front_tile(it):
                    """QKV matmuls, RMS-norm (rsqrt on DVE), RoPE for t-tile."""
                    qkv_ps = p1ps.tile([P, 512], F32, tag="qkv", name="qkv_ps")
                    for dk in range(4):
                        nc.tensor.matmul(
                            qkv_ps[:],
                            xt_sb[:, dk, ts(it, P)],
                            wqkv_sb[:, dk, :],
                            start=(dk == 0),
                            stop=(dk == 3),
                        )
                    # v (no norm): straight into v_sb with the ones-gap layout
                    nc.vector.tensor_copy(
                        v4[:, it, :, 0:HD],
                        qkv_ps[:, 384:512].rearrange("p (kv c) -> p kv c", c=HD),
                    )
                    # RMS stats over each 64-wide head strip (4 q + 2 k)
                    sq = p1sm.tile([P, 384], F32, tag="sq", name="sq")
                    nc.scalar.square(sq[:], qkv_ps[:, 0:384])
                    ms = p1sm.tile([P, 6], F32, tag="ms", name="ms")
                    nc.vector.reduce_sum(
                        ms[:], sq.rearrange("p (s c) -> p s c", c=HD), axis=AX.X
                    )
                    # rstd = rsqrt(ms/64 + eps): int bit-trick + 2 Newton steps,
                    # all on DVE. Keeping Sqrt/Ln off ACT means every ACT
                    # instruction uses the Exp table set (no table reloads).
                    nc.vector.tensor_scalar(
                        ms[:], ms[:], 1.0 / HD, EPS, op0=ALU.mult, op1=ALU.add
                    )
                    y = p1sm.tile([P, 6], F32, tag="y", name="y")
                    nc.vector.tensor_scalar(
                        y.bitcast(I32)[:], ms.bitcast(I32)[:], 1, None,
                        op0=ALU.arith_shift_right,
                    )
                    nc.vector.tensor_tensor(
                        y.bitcast(I32)[:], magic_sb[:], y.bitcast(I32)[:],
                        ALU.subtract,
                    )
                    nt_ = p1sm.tile([P, 6], F32, tag="nt", name="nt_")
                    for _ in range(2):
                        nc.vector.tensor_mul(nt_[:], y[:], y[:])
                        nc.vector.tensor_mul(nt_[:], nt_[:], ms[:])
                        nc.vector.tensor_scalar(
                            nt_[:], nt_[:], -0.5, 1.5, op0=ALU.mult, op1=ALU.add
                        )
                        nc.vector.tensor_mul(y[:], y[:], nt_[:])
                    # fold per-head gain * 1/sqrt(head_dim) into q strips
                    nc.vector.tensor_mul(y[:], y[:], gsc_sb[:])
                    qkn = p1sb.tile([P, 384], F32, tag="qkn", name="qkn")
                    qk3 = qkn.rearrange("p (s c) -> p s c", c=HD)
                    nc.vector.tensor_tensor(
                        qk3[:],
                        qkv_ps[:, 0:384].rearrange("p (s c) -> p s c", c=HD),
                        y[:, :, None].to_broadcast((P, 6, HD)),
                        ALU.mult,
                    )
                    # RoPE on first 16 dims of each strip (gpsimd)
                    x1 = qk3[:, :, 0:ROPE_HALF]
                    x2 = qk3[:, :, ROPE_HALF : 2 * ROPE_HALF]
                    cb = cs_sb[:, it, None, :].to_broadcast((P, 6, ROPE_HALF))
                    snb = sn_sb[:, it, None, :].to_broadcast((P, 6, ROPE_HALF))
                    t1 = p1sm.tile([P, 6, ROPE_HALF], F32, tag="t1", name="t1")
                    t2 = p1sm.tile([P, 6, ROPE_HALF], F32, tag="t2", name="t2")
                    nc.gpsimd.tensor_mul(t1[:], x1, snb)
                    nc.gpsimd.tensor_mul(x1, x1, cb)
                    nc.gpsimd.tensor_mul(t2[:], x2, snb)
                    nc.gpsimd.tensor_add(x1, x1, t2[:])
                    nc.gpsimd.tensor_mul(x2, x2, cb)
                    nc.gpsimd.tensor_sub(x2, x2, t1[:])
                    return qkn

                def emit_transposes(it, qkn):
                    for lo, dst in ((0, qTa), (128, qTb), (256, kT)):
                        tr = p1tr.tile([P, P], F32, tag="tr", name="tr")
                        nc.tensor.transpose(tr[:], qkn[:, lo : lo + P], ident[:])
                        nc.scalar.copy(dst[:, ts(it, P)], tr[:])

                def emit_attention_pair(qc, s_t):
                    """Both kv-halves (s_u=0 base 0, s_u=1 base 64) of one qT
                    tile, interleaved per kt block so the two score matmuls sit
                    adjacent in the PE stream (distinct row-groups -> they run
                    concurrently in the 128x128 array on hardware)."""
                    qT_tile = qTa if s_t == 0 else qTb
                    nkt = 4 * (qc + 1)
                    ctxs = []
                    for s_u in (0, 1):
                        ctxs.append(
                            cps.tile([P, 512], F32, tag=f"ctx{s_u}",
                                     name=f"ctx{s_u}", bufs=1)
                        )
                    for ktb in range(nkt):
                        d = ktb - 4 * qc
                        qlo = max(0, d) * P
                        pbs = []
                        for s_u in (0, 1):
                            kb = HD * s_u
                            q512 = qT_tile[kb : kb + HD, ts(qc, 512)]
                            s_ps = sps.tile([P, 512], F32, tag="s_ps", name="s_ps")
                            nc.tensor.matmul(
                                s_ps[:, qlo:512],
                                kT[kb : kb + HD, ts(ktb, P)],
                                q512[:, qlo:512],
                                start=True,
                                stop=True,
                            )
                            pb = pp.tile([P, 512], BF16, tag="pb", name="pb")
                            nc.scalar.activation(
                                pb[:, qlo:512], s_ps[:, qlo:512], AF.Exp
                            )
                            if d >= 0:
                                # zero the strictly-lower triangle (kt > q)
                                nc.gpsimd.affine_select(
                                    out=pb[:, qlo : qlo + P],
                                    in_=pb[:, qlo : qlo + P],
                                    compare_op=ALU.is_ge,
                                    fill=0.0,
                                    base=0,
                                    pattern=[[1, P]],
                                    channel_multiplier=-1,
                                )
                                if qlo > 0:
                                    nc.gpsimd.memset(pb[:, 0:qlo], 0.0)
                            pbs.append(pb)
                        for s_u in (0, 1):
                            nc.tensor.matmul(
                                ctxs[s_u][:],
                                v_sb[:, ktb, 128 * s_u : 128 * s_u + 128],
                                pbs[s_u][:],
                                start=(ktb == 0),
                                stop=(ktb == nkt - 1),
                            )
                    for s_u in (0, 1):
                        linv = lp.tile([HD, 512], F32, tag="linv", name="linv")
                        nc.vector.reciprocal(linv[:], ctxs[s_u][HD:128, :])
                        dst = ctxA if s_u == 0 else ctxB
                        rb = HD * s_t
                        nc.vector.tensor_tensor(
                            dst[rb : rb + HD, ts(qc, 512)],
                            ctxs[s_u][0:HD, :],
                            linv[:],
                            ALU.mult,
                        )

                def emit_proj(qc):
                    for ot in range(4):
                        y_ps = cps.tile(
                            [P, 512], F32, tag="y_ps", name="y_ps", bufs=1
                        )
                        for mt, src in ((0, ctxA), (1, ctxB)):
                            nc.tensor.matmul(
                                y_ps[:],
                                wp_sb[:, mt, ts(ot, P)],
                                src[:, ts(qc, 512)],
                                start=(mt == 0),
                                stop=(mt == 1),
                            )
                        y_sb = ysb.tile([P, 512], F32, tag="y_sb", name="y_sb")
                        nc.vector.tensor_copy(y_sb[:], y_ps[:])
                        nc.sync.dma_start(
                            yT[ot * P : (ot + 1) * P, qc * 512 : (qc + 1) * 512],
                            y_sb[:],
                        )

                do1 = 1 in phases
                do2 = 2 in phases
                do3 = 3 in phases
                for qc in range(NQC):
                    if do1:
                        qkns = []
                        for it in range(4 * qc, 4 * qc + 4):
                            qkns.append((it, emit_front_tile(it)))
                        for it, qkn in qkns:
                            emit_transposes(it, qkn)
                    if do2:
                        for s_t in (0, 1):
                            emit_attention_pair(qc, s_t)
                        if do3:
                            emit_proj(qc)

    nc.compile()
    return nc


def _host_inputs(x, w_q, w_k, w_v, w_proj, q_gain):
    """Build the 8 per-core input maps."""
    B = x.shape[0]
    inv_freq = 1.0 / (
        10000.0 ** (np.arange(0, 16, 2, dtype=np.float32) / np.float32(16.0))
    )
    freqs = np.outer(np.arange(T, dtype=np.float32), inv_freq)
    cs = np.cos(freqs).astype(np.float32)
    sn = np.sin(freqs).astype(np.float32)

    per_group = []
    for g in range(2):
        heads = [4 * g + 0, 4 * g + 2, 4 * g + 1, 4 * g + 3]
        qrows = np.concatenate([w_q[64 * h : 64 * h + 64] for h in heads], axis=0)
        krows = w_k[128 * g : 128 * g + 128]
        vrows = w_v[128 * g : 128 * g + 128]
        wqkvT = np.ascontiguousarray(
            np.concatenate([qrows, krows, vrows], axis=0).T.astype(np.float32)
        )
        wpT = np.ascontiguousarray(
            w_proj[:, 256 * g : 256 * g + 256].T.astype(np.float32)
        )
        gains = np.array(
            [q_gain[h] * 0.125 for h in heads] + [1.0, 1.0], dtype=np.float32
        )
        per_group.append((wqkvT, wpT, gains))

    in_maps = []
    for c in range(8):
        b, g = c // 2, c % 2
        wqkvT, wpT, gains = per_group[g]
        in_maps.append(
            {
                "xT": np.ascontiguousarray(x[b].T.astype(np.float32)),
                "wqkvT": wqkvT,
                "wpT": wpT,
                "cs": cs,
                "sn": sn,
                "gsc": gains,
            }
        )
    return in_maps


def kernel(x, w_q, w_k, w_v, w_proj, q_gain, _trace=False):
    x = np.asarray(x, dtype=np.float32)
    w_q = np.asarray(w_q, dtype=np.float32)
    w_k = np.asarray(w_k, dtype=np.float32)
    w_v = np.asarray(w_v, dtype=np.float32)
    w_proj = np.asarray(w_proj, dtype=np.float32)
    q_gain = np.asarray(q_gain, dtype=np.float32)

    if "nc" not in _CACHE:
        _CACHE["nc"] = _build()
    nc = _CACHE["nc"]

    in_maps = _host_inputs(x, w_q, w_k, w_v, w_proj, q_gain)
    res = bass_utils.run_bass_kernel_spmd(
        nc, in_maps, core_ids=list(range(8)), trace=_trace
    )
    _CACHE["last_result"] = res

    B = x.shape[0]
    y = np.empty((B, T, D), dtype=np.float32)
    for b in range(B):
        yT = res.results[2 * b]["yT"] + res.results[2 * b + 1]["yT"]
        y[b] = yT.T
    return y


# revision 43
# speedup vs baseline: 16125.1526x; 1.0182x over previous
"""Causal self-attention (GQA, QK-RMSNorm, partial RoPE, per-head gain) on 8 TRN2 cores.

Problem: B=4, T=2048, D=512; 8 q heads / 4 kv heads, head_dim 64, rope dims 16.

Sharding: core c handles batch b=c//2 and head-group g=c%2 (4 q heads + 2 kv
heads). Each core computes a partial projection y_part = ctx_g @ w_proj[:, g].T
over its 256 head dims; the host sums the two partials per batch.

On-device layout is fully "transposed" (feature-major) to keep every matmul
full-speed and avoid on-chip transposition of the attention probabilities:
  scores^T[kt, q] = k^T q   (kt on partitions, q on free axis)
  softmax without max-subtraction (|s| <= 40 << 88, exp cannot overflow fp32)
  ctx^T[d, q] accumulated as (v | ones)^T @ P  -- the 64 ones-columns make the
  matmul emit the softmax denominator replicated across 64 partitions for free.
Matmuls run in float32r (~1.5e-4 rel err, 4x faster than fp32 on TRN2);
the probabilities/V pair uses bf16, which enables the PE fast-weight-load
path for the per-block V weight loads. rsqrt for the RMS norm runs on the
vector engine (bit-trick + 2 Newton steps) so the scalar engine only ever
uses the Exp activation table (no table-reload stalls).
"""

import numpy as np

import concourse.bass as bass
import concourse.mybir as mybir
import concourse.tile as tile
from concourse import bacc, bass_utils
from concourse.bass import ts
from concourse.masks import make_identity

P = 128
T = 2048
D = 512
NT = T // P          # 16 t-tiles
NQC = T // 512       # 4 query chunks of 512
HD = 64
ROPE_HALF = 8
EPS = float(np.finfo(np.float32).eps)

F32 = mybir.dt.float32
F32R = mybir.dt.float32r
BF16 = mybir.dt.bfloat16
I32 = mybir.dt.int32
AX = mybir.AxisListType
AF = mybir.ActivationFunctionType
ALU = mybir.AluOpType

_CACHE = {}


def _build(phases=(1, 2, 3)):
    nc = bacc.Bacc("TRN2", target_bir_lowering=False, debug=False)

    xT = nc.dram_tensor("xT", [D, T], F32, kind="ExternalInput").ap()
    wqkvT = nc.dram_tensor("wqkvT", [D, 512], F32, kind="ExternalInput").ap()
    wpT = nc.dram_tensor("wpT", [256, 512], F32, kind="ExternalInput").ap()
    cs = nc.dram_tensor("cs", [T, ROPE_HALF], F32, kind="ExternalInput").ap()
    sn = nc.dram_tensor("sn", [T, ROPE_HALF], F32, kind="ExternalInput").ap()
    gsc = nc.dram_tensor("gsc", [6], F32, kind="ExternalInput").ap()
    yT = nc.dram_tensor("yT", [D, T], F32, kind="ExternalOutput").ap()

    with tile.TileContext(nc) as tc:
        with tc.tile_pool(name="persist", bufs=1) as persist:
            # weights/tables first: every front matmul depends on wqkv.
            # dk-sliced so the first accumulation matmul starts after 256KB.
            wqkv_sb = persist.tile([P, 4, 512], F32R)
            wqkvv = wqkvT.bitcast(F32R).rearrange("(o p) m -> p o m", p=P)
            nc.sync.dma_start(wqkv_sb[:, 0], wqkvv[:, 0])
            xt_sb = persist.tile([P, 4, T], F32R)
            xTv = xT.bitcast(F32R).rearrange("(o p) t -> p o t", p=P)
            nc.sync.dma_start(xt_sb[:, :, ts(0, P)], xTv[:, :, ts(0, P)])
            for dk in range(1, 4):
                nc.sync.dma_start(wqkv_sb[:, dk], wqkvv[:, dk])
            for it in range(1, 4):
                nc.sync.dma_start(
                    xt_sb[:, :, ts(it, P)], xTv[:, :, ts(it, P)]
                )
            cs_sb = persist.tile([P, NT, ROPE_HALF], F32)
            nc.sync.dma_start(cs_sb[:], cs.rearrange("(n p) f -> p n f", p=P))
            sn_sb = persist.tile([P, NT, ROPE_HALF], F32)
            nc.sync.dma_start(sn_sb[:], sn.rearrange("(n p) f -> p n f", p=P))
            gsc_sb = persist.tile([P, 6], F32)
            nc.sync.dma_start(gsc_sb[:], gsc[None, :].to_broadcast((P, 6)))
            for it in range(4, NT):
                nc.sync.dma_start(
                    xt_sb[:, :, ts(it, P)], xTv[:, :, ts(it, P)]
                )
            wp_sb = persist.tile([P, 2, 512], F32R)
            nc.sync.dma_start(
                wp_sb[:], wpT.bitcast(F32R).rearrange("(o p) m -> p o m", p=P)
            )
            ident = persist.tile([P, P], F32)
            make_identity(nc, ident[:])
            magic_sb = persist.tile([P, 6], I32)
            nc.vector.memset(magic_sb[:], 0x5F3759DF)

            # qTa: head0 (kv0) rows 0:64, head2 (kv1) rows 64:128
            # qTb: head1 (kv0) rows 0:64, head3 (kv1) rows 64:128
            qTa = persist.tile([P, T], F32R)
            qTb = persist.tile([P, T], F32R)
            kT = persist.tile([P, T], F32R)  # kv0 rows 0:64, kv1 rows 64:128
            # v_sb per t-tile columns: [v_kv0(64) | ones(64) | v_kv1(64) | ones(64)]
            # bf16: the AV matmul weight-loads v every block; bf16 enables the
            # PE fast-weight-load path (4x faster LDWEIGHTS on hardware)
            v_sb = persist.tile([P, NT, 256], BF16)
            v4 = v_sb.rearrange("p n (kv c) -> p n kv c", kv=2)
            nc.gpsimd.memset(v4[:, :, :, HD:128], 1.0)
            # ctxA: heads (4g+0, 4g+1); ctxB: heads (4g+2, 4g+3) -- matches wpT rows
            ctxA = persist.tile([P, T], F32R)
            ctxB = persist.tile([P, T], F32R)

            # ---- Software-pipelined schedule, per 512-token chunk qc:
            #   front(qc): QKV matmuls + RMS stats + rsqrt + mulback + RoPE
            #   attention(qc-1) first half   (fills PE/ACT while rope settles)
            #   transposes(qc)
            #   attention(qc-1) second half + projection(qc-1)
            with (
                tc.tile_pool(name="p1ps", bufs=2, space="PSUM") as p1ps,
                tc.tile_pool(name="p1tr", bufs=1, space="PSUM") as p1tr,
                tc.tile_pool(name="sps", bufs=2, space="PSUM") as sps,
                tc.tile_pool(name="cps", bufs=2, space="PSUM") as cps,
                tc.tile_pool(name="p1sb", bufs=6) as p1sb,
                tc.tile_pool(name="p1sm", bufs=8) as p1sm,
                tc.tile_pool(name="pp", bufs=8) as pp,
                tc.tile_pool(name="lp", bufs=4) as lp,
                tc.tile_pool(name="ysb", bufs=4) as ysb,
            ):

                def emit_front_tile(it):
                    """QKV matmuls, RMS-norm (rsqrt on DVE), RoPE for t-tile."""
                    qkv_ps = p1ps.tile([P, 512], F32, tag="qkv", name="qkv_ps")
                    for dk in range(4):
                        nc.tensor.matmul(
                            qkv_ps[:],
                            xt_sb[:, dk, ts(it, P)],
                            wqkv_sb[:, dk, :],
                            start=(dk == 0),
                            stop=(dk == 3),
                        )
                    # v (no norm): straight into v_sb with the ones-gap layout
                    nc.vector.tensor_copy(
                        v4[:, it, :, 0:HD],
                        qkv_ps[:, 384:512].rearrange("p (kv c) -> p kv c", c=HD),
                    )
                    # RMS stats over each 64-wide head strip (4 q + 2 k)
                    sq = p1sm.tile([P, 384], F32, tag="sq", name="sq")
                    nc.scalar.square(sq[:], qkv_ps[:, 0:384])
                    ms = p1sm.tile([P, 6], F32, tag="ms", name="ms")
                    nc.vector.reduce_sum(
                        ms[:], sq.rearrange("p (s c) -> p s c", c=HD), axis=AX.X
                    )
                    # rstd = rsqrt(ms/64 + eps): int bit-trick + 2 Newton steps,
                    # all on DVE. Keeping Sqrt/Ln off ACT means every ACT
                    # instruction uses the Exp table set (no table reloads).
                    nc.vector.tensor_scalar(
                        ms[:], ms[:], 1.0 / HD, EPS, op0=ALU.mult, op1=ALU.add
                    )
                    y = p1sm.tile([P, 6], F32, tag="y", name="y")
                    nc.vector.tensor_scalar(
                        y.bitcast(I32)[:], ms.bitcast(I32)[:], 1, None,
                        op0=ALU.arith_shift_right,
                    )
                    nc.vector.tensor_tensor(
                        y.bitcast(I32)[:], magic_sb[:], y.bitcast(I32)[:],
                        ALU.subtract,
                    )
                    nt_ = p1sm.tile([P, 6], F32, tag="nt", name="nt_")
                    for _ in range(2):
                        nc.vector.tensor_mul(nt_[:], y[:], y[:])
                        nc.vector.tensor_mul(nt_[:], nt_[:], ms[:])
                        nc.vector.tensor_scalar(
                            nt_[:], nt_[:], -0.5, 1.5, op0=ALU.mult, op1=ALU.add
                        )
                        nc.vector.tensor_mul(y[:], y[:], nt_[:])
                    # fold per-head gain * 1/sqrt(head_dim) into q strips
                    nc.vector.tensor_mul(y[:], y[:], gsc_sb[:])
                    qkn = p1sb.tile([P, 384], F32, tag="qkn", name="qkn")
                    qk3 = qkn.rearrange("p (s c) -> p s c", c=HD)
                    nc.vector.tensor_tensor(
                        qk3[:],
                        qkv_ps[:, 0:384].rearrange("p (s c) -> p s c", c=HD),
                        y[:, :, None].to_broadcast((P, 6, HD)),
                        ALU.mult,
                    )
                    # RoPE on first 16 dims of each strip (gpsimd)
                    x1 = qk3[:, :, 0:ROPE_HALF]
                    x2 = qk3[:, :, ROPE_HALF : 2 * ROPE_HALF]
                    cb = cs_sb[:, it, None, :].to_broadcast((P, 6, ROPE_HALF))
                    snb = sn_sb[:, it, None, :].to_broadcast((P, 6, ROPE_HALF))
                    t1 = p1sm.tile([P, 6, ROPE_HALF], F32, tag="t1", name="t1")
                    t2 = p1sm.tile([P, 6, ROPE_HALF], F32, tag="t2", name="t2")
                    nc.gpsimd.tensor_mul(t1[:], x1, snb)
                    nc.gpsimd.tensor_mul(x1, x1, cb)
                    nc.gpsimd.tensor_mul(t2[:], x2, snb)
                    nc.gpsimd.tensor_add(x1, x1, t2[:])
                    nc.gpsimd.tensor_mul(x2, x2, cb)
                    nc.gpsimd.tensor_sub(x2, x2, t1[:])
                    return qkn

                def emit_transposes(it, qkn):
                    for lo, dst in ((0, qTa), (128, qTb), (256, kT)):
                        tr = p1tr.tile([P, P], F32, tag="tr", name="tr")
                        nc.tensor.transpose(tr[:], qkn[:, lo : lo + P], ident[:])
                        nc.scalar.copy(dst[:, ts(it, P)], tr[:])

                def emit_attention_pair(qc, s_t):
                    """Both kv-halves (s_u=0 base 0, s_u=1 base 64) of one qT
                    tile, interleaved per kt block so the two score matmuls sit
                    adjacent in the PE stream (distinct row-groups -> they run
                    concurrently in the 128x128 array on hardware)."""
                    qT_tile = qTa if s_t == 0 else qTb
                    nkt = 4 * (qc + 1)
                    ctxs = []
                    for s_u in (0, 1):
                        ctxs.append(
                            cps.tile([P, 512], F32, tag=f"ctx{s_u}",
                                     name=f"ctx{s_u}", bufs=1)
                        )
                    for ktb in range(nkt):
                        d = ktb - 4 * qc
                        qlo = max(0, d) * P
                        pbs = []
                        for s_u in (0, 1):
                            kb = HD * s_u
                            q512 = qT_tile[kb : kb + HD, ts(qc, 512)]
                            s_ps = sps.tile([P, 512], F32, tag="s_ps", name="s_ps")
                            nc.tensor.matmul(
                                s_ps[:, qlo:512],
                                kT[kb : kb + HD, ts(ktb, P)],
                                q512[:, qlo:512],
                                start=True,
                                stop=True,
                            )
                            pb = pp.tile([P, 512], BF16, tag="pb", name="pb")
                            nc.scalar.activation(
                                pb[:, qlo:512], s_ps[:, qlo:512], AF.Exp
                            )
                            if d >= 0:
                                # zero the strictly-lower triangle (kt > q)
                                nc.gpsimd.affine_select(
                                    out=pb[:, qlo : qlo + P],
                                    in_=pb[:, qlo : qlo + P],
                                    compare_op=ALU.is_ge,
                                    fill=0.0,
                                    base=0,
                                    pattern=[[1, P]],
                                    channel_multiplier=-1,
                                )
                                if qlo > 0:
                                    nc.gpsimd.memset(pb[:, 0:qlo], 0.0)
                            pbs.append(pb)
                        for s_u in (0, 1):
                            nc.tensor.matmul(
                                ctxs[s_u][:],
                                v_sb[:, ktb, 128 * s_u : 128 * s_u + 128],
                                pbs[s_u][:],
                                start=(ktb == 0),
                                stop=(ktb == nkt - 1),
                            )
                    for s_u in (0, 1):
                        linv = lp.tile([HD, 512], F32, tag="linv", name="linv")
                        nc.vector.reciprocal(linv[:], ctxs[s_u][HD:128, :])
                        dst = ctxA if s_u == 0 else ctxB
                        rb = HD * s_t
                        nc.vector.tensor_tensor(
                            dst[rb : rb + HD, ts(qc, 512)],
                            ctxs[s_u][0:HD, :],
                            linv[:],
                            ALU.mult,
                        )

                def emit_proj(qc):
                    for ot in range(4):
                        y_ps = cps.tile(
                            [P, 512], F32, tag="y_ps", name="y_ps", bufs=1
                        )
                        for mt, src in ((0, ctxA), (1, ctxB)):
                            nc.tensor.matmul(
                                y_ps[:],
                                wp_sb[:, mt, ts(ot, P)],
                                src[:, ts(qc, 512)],
                                start=(mt == 0),
                                stop=(mt == 1),
                            )
                        y_sb = ysb.tile([P, 512], F32, tag="y_sb", name="y_sb")
                        nc.vector.tensor_copy(y_sb[:], y_ps[:])
                        nc.sync.dma_start(
                            yT[ot * P : (ot + 1) * P, qc * 512 : (qc + 1) * 512],
                            y_sb[:],
                        )

                do1 = 1 in phases
                do2 = 2 in phases
                do3 = 3 in phases
                for qc in range(NQC):
                    if do1:
                        qkns = []
                        for it in range(4 * qc, 4 * qc + 4):
                            qkns.append((it, emit_front_tile(it)))
                        for it, qkn in qkns:
                            emit_transposes(it, qkn)
                    if do2:
                        for s_t in (0, 1):
                            emit_attention_pair(qc, s_t)
                        if do3:
                            emit_proj(qc)

    nc.compile()
    return nc


def _host_inputs(x, w_q, w_k, w_v, w_proj, q_gain):
    """Build the 8 per-core input maps."""
    B = x.shape[0]
    inv_freq = 1.0 / (
        10000.0 ** (np.arange(0, 16, 2, dtype=np.float32) / np.float32(16.0))
    )
    freqs = np.outer(np.arange(T, dtype=np.float32), inv_freq)
    cs = np.cos(freqs).astype(np.float32)
    sn = np.sin(freqs).astype(np.float32)

    per_group = []
    for g in range(2):
        heads = [4 * g + 0, 4 * g + 2, 4 * g + 1, 4 * g + 3]
        qrows = np.concatenate([w_q[64 * h : 64 * h + 64] for h in heads], axis=0)
        krows = w_k[128 * g : 128 * g + 128]
        vrows = w_v[128 * g : 128 * g + 128]
        wqkvT = np.ascontiguousarray(
            np.concatenate([qrows, krows, vrows], axis=0).T.astype(np.float32)
        )
        wpT = np.ascontiguousarray(
            w_proj[:, 256 * g : 256 * g + 256].T.astype(np.float32)
        )
        gains = np.array(
            [q_gain[h] * 0.125 for h in heads] + [1.0, 1.0], dtype=np.float32
        )
        per_group.append((wqkvT, wpT, gains))

    in_maps = []
    for c in range(8):
        b, g = c // 2, c % 2
        wqkvT, wpT, gains = per_group[g]
        in_maps.append(
            {
                "xT": np.ascontiguousarray(x[b].T.astype(np.float32)),
                "wqkvT": wqkvT,
                "wpT": wpT,
                "cs": cs,
                "sn": sn,
                "gsc": gains,
            }
        )
    return in_maps


def kernel(x, w_q, w_k, w_v, w_proj, q_gain, _trace=False):
    x = np.asarray(x, dtype=np.float32)
    w_q = np.asarray(w_q, dtype=np.float32)
    w_k = np.asarray(w_k, dtype=np.float32)
    w_v = np.asarray(w_v, dtype=np.float32)
    w_proj = np.asarray(w_proj, dtype=np.float32)
    q_gain = np.asarray(q_gain, dtype=np.float32)

    if "nc" not in _CACHE:
        _CACHE["nc"] = _build()
    nc = _CACHE["nc"]

    in_maps = _host_inputs(x, w_q, w_k, w_v, w_proj, q_gain)
    res = bass_utils.run_bass_kernel_spmd(
        nc, in_maps, core_ids=list(range(8)), trace=_trace
    )
    _CACHE["last_result"] = res

    B = x.shape[0]
    y = np.empty((B, T, D), dtype=np.float32)
    for b in range(B):
        yT = res.results[2 * b]["yT"] + res.results[2 * b + 1]["yT"]
        y[b] = yT.T
    return y
